# revision 2
# baseline (speedup 1.0000x reference)
"""Trainium2 Bass kernel for the Elman-RNN place-cell problem, v2.

Parity-split parallelization: the two batch halves (128 each) are
independent recurrences, so even cores {0,2,4,6} own batch half A and odd
cores {1,3,5,7} own half B.  Core c computes hidden columns
[1024*(c//2), 1024*(c//2+1)) of its half; one 4-rank AllGather per step
over the parity group ([[0,2,4,6],[1,3,5,7]]) rebuilds the full hidden
state.  Wire traffic: 768KB/rank/step vs 1.75MB for the v1 8-rank scheme.
The shard is produced and gathered in two column chunks so the second
half of the next step's contraction can overlap the second AG.

The input projection x_t = v_t @ W_in runs on-device as K=2 matmuls that
seed the PSUM accumulators (start=True), so no per-step x DMA and no
vector add; relu is a single tensor_scalar_max PSUM->SBUF per bank.  The
decode (g @ W_dec) runs at full width: core c decodes output block c//2
(128 cols of NP) for its half, filling the AllGather wait.
"""
import os
import sys
import functools

sys.path.insert(0, "/opt/trn_rl_repo")

import numpy as np

from concourse import bass, bacc, mybir, tile  # noqa: E402
from concourse import bass_utils  # noqa: E402

B = 256  # noqa: E305
T = 100
NG = 4096
NP = 512
NCORES = 8
NGRP = 4                  # ranks per parity group
SC = NG // NGRP           # 1024 hidden cols per core
MT = SC // 128            # 8 output m-tiles per core
KT = NG // 128            # 32 contraction k-tiles
BH = B // 2               # 128 batch per half
M_CHUNKS = ((0, 1, 2, 3), (4, 5, 6, 7))
FP = mybir.dt.float32
BF = mybir.dt.bfloat16

RGP = [[0, 2, 4, 6], [1, 3, 5, 7]]


def _build(t_steps=T):
    nc = bacc.Bacc("TRN2", target_bir_lowering=False, debug=False,
                   num_devices=NCORES)
    # weight rows (contraction) in k-tile order (ch, j, mj):
    #   g = j*1024 + (ch*4 + mj)*128 + p
    wrec = nc.dram_tensor("wrec", [128, KT * MT * 128], BF,
                          kind="ExternalInput")
    wdec = nc.dram_tensor("wdec", [128, KT * 128], BF, kind="ExternalInput")
    win = nc.dram_tensor("win", [2, MT * 128], BF, kind="ExternalInput")
    vin = nc.dram_tensor("v", [2, t_steps * BH], BF, kind="ExternalInput")
    h0 = nc.dram_tensor("h0", [128, NGRP * MT * BH], BF,
                        kind="ExternalInput")
    out = nc.dram_tensor("out", [t_steps, 128, BH], FP,
                         kind="ExternalOutput")

    with tile.TileContext(nc) as tc:
        with tc.tile_pool(name="wpool", bufs=1) as wpool, \
             tc.tile_pool(name="hpool", bufs=2) as hpool, \
             tc.tile_pool(name="hnpool", bufs=2) as hnpool, \
             tc.tile_pool(name="decpool", bufs=2) as decpool, \
             tc.tile_pool(name="psr", bufs=1, space="PSUM") as psr, \
             tc.tile_pool(name="psd", bufs=2, space="PSUM") as psd, \
             tc.tile_pool(name="dram_i", bufs=3, space="DRAM") as dram_i, \
             tc.tile_pool(name="dram_o", bufs=3, space="DRAM") as dram_o:

            wrec_sb = wpool.tile([128, KT, MT, 128], BF, name="wrec_sb")
            nc.scalar.dma_start(
                out=wrec_sb[:],
                in_=wrec[:].rearrange("p (k m q) -> p k m q", k=KT, m=MT))
            wdec_sb = wpool.tile([128, KT, 128], BF, name="wdec_sb")
            nc.scalar.dma_start(
                out=wdec_sb[:],
                in_=wdec[:].rearrange("p (k q) -> p k q", k=KT))
            win_sb = wpool.tile([2, MT, 128], BF, name="win_sb")
            nc.scalar.dma_start(
                out=win_sb[:], in_=win[:].rearrange("d (m q) -> d m q", m=MT))
            v_sb = wpool.tile([2, t_steps, BH], BF, name="v_sb")
            nc.scalar.dma_start(
                out=v_sb[:], in_=vin[:].rearrange("d (t b) -> d t b",
                                                  t=t_steps))

            # gathered hidden state, chunk ch: [p, j(group rank), mj, b]
            # where mj indexes M_CHUNKS[ch]
            def new_hts(ch):
                return hpool.tile([128, NGRP, len(M_CHUNKS[ch]), BH], BF,
                                  name=f"hts{ch}", tag=f"hts{ch}")

            hts = [new_hts(0), new_hts(1)]
            h0r = h0[:].rearrange("p (j m b) -> p j m b", j=NGRP, m=MT)
            nc.scalar.dma_start(
                out=hts[0][:], in_=h0r[:, :, M_CHUNKS[0][0]:M_CHUNKS[0][-1] + 1])
            nc.scalar.dma_start(
                out=hts[1][:], in_=h0r[:, :, M_CHUNKS[1][0]:M_CHUNKS[1][-1] + 1])

            def recur_phase(ms, kch, i, seed, close=False):
                # x-seed the psum banks (start=True zeroes the bank)
                if seed:
                    for m in ms:
                        nc.tensor.matmul(
                            pss[m // 2][:, m % 2, :],
                            win_sb[:, m, :],
                            v_sb[:, i, :],
                            start=(m % 2 == 0), stop=False,
                            skip_group_check=True)
                kms = M_CHUNKS[kch]
                for j in range(NGRP):
                    for mj, km in enumerate(kms):
                        k = j * MT + km
                        last = close and j == NGRP - 1 and mj == len(kms) - 1
                        for m in ms:
                            nc.tensor.matmul(
                                pss[m // 2][:, m % 2, :],
                                wrec_sb[:, k, m, :],
                                hts[kch][:, j, mj, :],
                                start=False, stop=last,
                                skip_group_check=True)

            def decode(srcs, t):
                dec_sb = decpool.tile([128, BH], FP, name="dec_sb",
                                      tag="dec_sb")
                ps = psd.tile([128, BH], FP, name="ps_dec", tag="ps_dec")
                first = True
                for ch in range(2):
                    for j in range(NGRP):
                        for mj, km in enumerate(M_CHUNKS[ch]):
                            k = j * MT + km
                            nc.tensor.matmul(
                                ps[:],
                                wdec_sb[:, k, :],
                                srcs[ch][:, j, mj, :],
                                start=first,
                                stop=(ch == 1 and j == NGRP - 1
                                      and km == MT - 1),
                                skip_group_check=True)
                            first = False
                nc.vector.tensor_copy(dec_sb[:], ps[:])
                nc.scalar.dma_start(out=out[t], in_=dec_sb[:])

            def send(ch):
                ms = M_CHUNKS[ch]
                nmc = len(ms)
                hn = hnpool.tile([128, nmc, BH], BF, name=f"hn{ch}",
                                 tag=f"hn{ch}")
                for p in range(nmc // 2):
                    bank = pss[ms[0] // 2 + p]
                    nc.vector.tensor_scalar_max(
                        hn[:, 2 * p:2 * p + 2, :], bank[:], 0.0)
                cc_i = dram_i.tile([128, nmc, BH], BF, name=f"cc_i{ch}",
                                   tag=f"cc_i{ch}")
                cc_o = dram_o.tile([NGRP, 128, nmc, BH], BF,
                                   name=f"cc_o{ch}", tag=f"cc_o{ch}")
                nc.sync.dma_start(out=cc_i[:], in_=hn[:])
                nc.gpsimd.collective_compute(
                    "AllGather", mybir.AluOpType.bypass,
                    replica_groups=RGP,
                    ins=[cc_i[:].opt()], outs=[cc_o[:].opt()])
                nh = new_hts(ch)
                for c in range(2):
                    nc.sync.dma_start(
                        out=nh[:, 2 * c:2 * c + 2],
                        in_=cc_o[2 * c:2 * c + 2].rearrange(
                            "j p m b -> p j m b"))
                return nh

            for i in range(t_steps):
                pss = [psr.tile([128, 2, BH], FP, name=f"ps{p}",
                                tag=f"ps{p}") for p in range(4)]
                prev = [hts[0], hts[1]]
                new = [None, None]
                for ch in range(2):
                    recur_phase(M_CHUNKS[ch], 0, i, seed=True)
                    recur_phase(M_CHUNKS[ch], 1, i, seed=False, close=True)
                    new[ch] = send(ch)
                    if ch == 0 and i >= 1:
                        decode(prev, i - 1)
                hts = new

            decode(hts, t_steps - 1)

    nc.compile()
    return nc


@functools.lru_cache(maxsize=1)
def _built():
    return _build()


def _to_bf(a):
    import ml_dtypes
    return np.ascontiguousarray(a).astype(ml_dtypes.bfloat16)


def _prep_inputs(v, P0, W_enc, W_in, W_rec, W_dec, t_steps=T):
    v = np.asarray(v, np.float32)
    P0 = np.asarray(P0, np.float32)
    W_enc = np.asarray(W_enc, np.float32)
    W_in = np.asarray(W_in, np.float32)
    W_rec = np.asarray(W_rec, np.float32)
    W_dec = np.asarray(W_dec, np.float32)

    h0f = P0 @ W_enc                       # [B, NG]
    # contraction row order (as k-tiles): k=(j, ch*4+mj): rows of W_* are
    # grouped [j, m(=ch*4+mj), p]: plain reshape of the natural g order.
    wr = W_rec.reshape(NGRP, MT, 128, NG)  # [j, m, p, gout]
    wd = W_dec.reshape(NGRP, MT, 128, NP)  # [j, m, p, np]

    in_maps = []
    for c in range(NCORES):
        e = c % 2          # batch half
        gi = c // 2        # column group [1024*gi : 1024*(gi+1))
        # k-tile enumeration used on-device: k = j*MT + m, i.e. plain
        # (j, m) order -> natural row order. cols = my shard.
        wrec_core = wr[:, :, :, gi * SC:(gi + 1) * SC]      # [4,8,128,1024]
        wrec_core = wrec_core.transpose(2, 0, 1, 3)         # [p,j,m,1024]
        wrec_core = np.ascontiguousarray(wrec_core).reshape(
            128, KT * MT * 128)
        wdec_core = wd[:, :, :, gi * 128:(gi + 1) * 128]    # [4,8,128,128]
        wdec_core = wdec_core.transpose(2, 0, 1, 3)         # [p,j,m,128]
        wdec_core = np.ascontiguousarray(wdec_core).reshape(128, KT * 128)
        win_core = np.ascontiguousarray(W_in[:, gi * SC:(gi + 1) * SC])
        v_core = np.ascontiguousarray(
            v[e * BH:(e + 1) * BH].transpose(2, 1, 0)).reshape(
                2, t_steps * BH)
        # h0 for my half in layout [p, j, m, b]
        h0h = h0f[e * BH:(e + 1) * BH].T                    # [NG, BH]
        h0h = h0h.reshape(NGRP, MT, 128, BH)                # [j,m,p,b]
        h0h = h0h.transpose(2, 0, 1, 3)                     # [p,j,m,b]
        h0h = np.ascontiguousarray(h0h).reshape(128, NGRP * MT * BH)
        in_maps.append({
            "wrec": _to_bf(wrec_core),
            "wdec": _to_bf(wdec_core),
            "win": _to_bf(win_core),
            "v": _to_bf(v_core),
            "h0": _to_bf(h0h),
        })
    return in_maps


def _assemble(results, t_steps=T):
    full = np.empty((B, t_steps, NP), np.float32)
    for c in range(NCORES):
        a = results[c]["out"]          # [T, np_col 128, b 128]
        e = c % 2
        gi = c // 2
        full[e * BH:(e + 1) * BH, :, gi * 128:(gi + 1) * 128] = \
            a.transpose(2, 0, 1)
    return full


last_exec_time_ns = None


def kernel(v, P0, W_enc, W_in, W_rec, W_dec):
    global last_exec_time_ns
    nc = _built()
    in_maps = _prep_inputs(v, P0, W_enc, W_in, W_rec, W_dec)

    trace = bool(int(os.environ.get("RNN_TRACE", "0")))
    if trace:
        try:
            import types
            sys.path.insert(0, "/root/.axon_site")
            from trn_agent_boot.trn_boot import _ntff_profile_via_ctypes
            import antenv  # noqa: F401
            if "antenv.axon_hooks" not in sys.modules:
                mod = types.ModuleType("antenv.axon_hooks")
                hook = _ntff_profile_via_ctypes("/opt/axon/libaxon_pjrt.so")
                mod.get_axon_ntff_profile_hook = lambda: hook
                sys.modules["antenv.axon_hooks"] = mod
        except Exception as e:  # pragma: no cover
            print("trace shim failed:", e)

    last_err = None
    for _ in range(3):
        try:
            res = bass_utils.run_bass_kernel_spmd(
                nc, in_maps, core_ids=list(range(NCORES)), trace=trace)
            last_exec_time_ns = res.exec_time_ns
            return _assemble(res.results)
        except Exception as e:  # pragma: no cover
            last_err = e
            import time
            time.sleep(5)
    raise last_err
